# revision 39
# baseline (speedup 1.0000x reference)
"""LDPC encoder kernel for Trainium2 (8 NeuronCores, batch-sharded).

Computes out = 1 - 2*((m @ G^T) mod 2)  (BPSK-mapped LDPC codeword).

  m: [16384, 1200] int32 (0/1)   G: [2400, 1200] float32 (0/1)
  out: [16384, 2400] float32 (+-1)

Strategy:
  - Shard the batch over 8 cores (2048 rows each); G replicated.
  - G is systematic (G[:1200] == I), so out[:, :1200] = 1 - 2*m is a pure
    elementwise map (done on the otherwise-idle GPSIMD engine); only
    the 1200 parity columns need a matmul.
  - Matmul in fp8e4 (values 0/1 are exact; PSUM accumulates fp32
    exactly, psum = d = bit count). Host feeds m transposed ([K,B]
    layout) so the stationary operand needs no on-device transpose.
    Parity+BPSK post-processing (cast psum f32->i16, p = &1,
    out = 1 - 2*p) is spread across DVE/ACT/GPSIMD so no single engine
    exceeds the PE's per-tile cadence. (DVE/GPSIMD have no mod op, and
    ACT Sin has no range reduction -- both verified on HW.)
  - Output written as fp8e4 (+-1 exact), cast to f32 on host.
  - USE_DR=True packs contraction rows in pairs (MatmulPerfMode.DoubleRow)
    for ~1.4x tensor-engine throughput.
"""

import numpy as np
import ml_dtypes

BF16 = ml_dtypes.bfloat16

B_FULL = 16384
K_MSG = 1200
N_BITS = 2400
N_CORES = 8
B_LOC = B_FULL // N_CORES  # 2048
K_PAD = 1280  # zero-padded to 5 DoubleRow k-pair-tiles of 256
P = 128

_CACHE: dict = {}
USE_DR = True


def _mm_np_dtype():
    import concourse.mybir as mybir
    return mybir.dt.np(mybir.dt.float8e4)


def _build(bl, k_msg, k_pad, n_par, n_bits, base_col, with_identity,
           use_dr=False):
    """Build + compile the per-core Bass program.

    bl: local batch rows; n_par: matmul output columns; base_col: where the
    matmul columns land in the output; with_identity: also emit
    out[:, :k_msg] = 1-2*m from a natural-layout copy of m.
    """
    import concourse.bacc as bacc
    import concourse.mybir as mybir
    import concourse.tile as tile

    f32 = mybir.dt.float32
    i16 = mybir.dt.int16
    fp8 = mybir.dt.float8e4
    Alu = mybir.AluOpType
    Act = mybir.ActivationFunctionType

    nc = bacc.Bacc("TRN2", target_bir_lowering=False, debug=False,
                   num_devices=N_CORES)

    k_step = 2 * P if use_dr else P
    kt_n = k_pad // k_step
    if use_dr:
        # paired layout: DRAM row (t*P + p) = concat(x[2P*t + p], x[2P*t + P + p])
        mT = nc.dram_tensor("mT", [kt_n * P, 2 * bl], fp8, kind="ExternalInput")
        gT = nc.dram_tensor("GT2", [kt_n * P, 2 * n_par], fp8,
                            kind="ExternalInput")
    else:
        mT = nc.dram_tensor("mT", [k_pad, bl], fp8, kind="ExternalInput")
        gT = nc.dram_tensor("GT2", [k_pad, n_par], fp8, kind="ExternalInput")
    out = nc.dram_tensor("out", [bl, n_bits], fp8, kind="ExternalOutput")
    mnat = None
    if with_identity:
        mnat = nc.dram_tensor("mnat", [bl, k_msg], fp8, kind="ExternalInput")

    nb = bl // P
    chunks = []
    n0 = 0
    while n0 < n_par:
        w = min(512, n_par - n0)
        chunks.append((n0, w))
        n0 += w

    with tile.TileContext(nc) as tc:
        with (
            tc.tile_pool(name="const", bufs=1) as cpool,
            tc.tile_pool(name="mn", bufs=4) as mnpool,
            tc.tile_pool(name="po", bufs=6) as popool,
            tc.tile_pool(name="io", bufs=4) as iopool,
            tc.tile_pool(name="ps", bufs=8, space="PSUM") as pspool,
        ):
            # Warm up the PE clock during the DMA prologue: the HAM clock
            # gate starts at 1.2GHz and needs ~3.4us of sustained activity
            # to release to 2.4GHz. ~48 dummy matmuls on a memset tile keep
            # the PE busy while the const loads stream, so the real matmuls
            # start (and stay) at full clock instead of ramping mid-kernel.
            wt = cpool.tile([P, 64], fp8, tag="warm")
            nc.vector.memset(wt[:], 1.0)
            wps = pspool.tile([P, 512], f32, tag="ps", name="warmps")
            for _ in range(48):
                nc.tensor.matmul(wps[0:64, 0:64], wt[:], wt[:],
                                 start=True, stop=True)

            # const loads split across both HWDGE rings: mt (big) on sync,
            # gt on scalar -- the rings transfer concurrently (~360GB/s
            # each). (Alternating mt/gt across rings measured WORSE: it
            # delays the last k-tiles and grows the ramp gaps 3.9->5.1us.)
            gts, mts = [], []
            for t in range(kt_n):
                rs = slice(t * P, (t + 1) * P)
                if use_dr:
                    gt_t = cpool.tile([P, 2, n_par], fp8, tag=f"gt{t}")
                    nc.scalar.dma_start(out=gt_t[:], in_=gT[rs, :])
                    mt_t = cpool.tile([P, 2, bl], fp8, tag=f"mt{t}")
                    nc.sync.dma_start(out=mt_t[:], in_=mT[rs, :])
                else:
                    gt_t = cpool.tile([P, n_par], fp8, tag=f"gt{t}")
                    nc.scalar.dma_start(out=gt_t[:], in_=gT[rs, :])
                    mt_t = cpool.tile([P, bl], fp8, tag=f"mt{t}")
                    nc.sync.dma_start(out=mt_t[:], in_=mT[rs, :])
                gts.append(gt_t)
                mts.append(mt_t)

            # The first two b-tiles are processed as one group so the PE has
            # ~2x the work per arriving k-pair-tile while the const loads
            # are still streaming (otherwise it idles ~0.85us per k-tile
            # during the ramp). Later b-tiles run singly (the const tiles
            # are resident by then, and single groups keep PSUM lookahead).
            # (A 3-wide first group measured WORSE: the 9th PSUM tile's
            # warmup-bank dependency and the 3-tile post backlog cost more
            # than the extra ramp coverage gains.)
            pair_ok = nb >= 2 and 2 * len(chunks) + 1 <= 8  # PSUM banks
            groups = (([[0, 1]] + [[b] for b in range(2, nb)]) if pair_ok
                      else [[b] for b in range(nb)])
            for grp in groups:
                pst_map = {
                    b: [pspool.tile([P, 512], f32, tag="ps",
                                    name=f"ps{b}_{ci}")
                        for ci in range(len(chunks))]
                    for b in grp
                }
                for t in range(kt_n):
                    for b in grp:
                        bs = slice(b * P, (b + 1) * P)
                        for ci, (n0, w) in enumerate(chunks):
                            if use_dr:
                                nc.tensor.matmul(
                                    pst_map[b][ci][:, :w],
                                    mts[t][:, :, bs],
                                    gts[t][:, :, n0:n0 + w],
                                    start=(t == 0),
                                    stop=(t == kt_n - 1),
                                    perf_mode=mybir.MatmulPerfMode.DoubleRow,
                                )
                            else:
                                nc.tensor.matmul(
                                    pst_map[b][ci][:, :w],
                                    mts[t][:, bs],
                                    gts[t][:, n0:n0 + w],
                                    start=(t == 0),
                                    stop=(t == kt_n - 1),
                                )
                # all 2400 output cols assembled in one fp8 buffer, one DMA.
                # post chain per chunk: cast f32->i16, p = &1, out = 1-2p.
                # Work is spread so every engine stays under the PE's
                # ~2.7us/b-tile cadence (DVE alone would be co-critical).
                for b in grp:
                    bs = slice(b * P, (b + 1) * P)
                    psts = pst_map[b]
                    ob = iopool.tile([P, n_bits], fp8, tag="ob",
                                     name=f"ob{b}")
                    if with_identity:
                        # identity map first: it has no psum dependency, and
                        # GPSIMD's queue is strict FIFO -- emitted after the
                        # parity affine it would wait on the last matmuls
                        mn = mnpool.tile([P, k_msg], fp8, tag="mn")
                        # sync ring, NOT scalar: a DMA trigger blocks its
                        # issuing engine's queue while waiting on pool
                        # rotation, and the Scalar engine also runs the
                        # parity ACTIVATEs (moving mnat there cost +11us)
                        nc.sync.dma_start(out=mn[:], in_=mnat[bs, :])
                        nc.gpsimd.tensor_scalar(
                            ob[:, 0:k_msg], mn[:], -2.0, 1.0,
                            op0=Alu.mult, op1=Alu.add,
                        )
                    for ci, (n0, w) in enumerate(chunks):
                        it = popool.tile([P, 512], i16, tag="pi",
                                         name=f"pi{b}_{ci}")
                        if ci % 3 == 2:
                            nc.scalar.activation(
                                it[:, :w], psts[ci][:, :w], Act.Copy,
                            )
                        else:
                            nc.vector.tensor_copy(it[:, :w], psts[ci][:, :w])
                        pt = popool.tile([P, 512], i16, tag="pp",
                                         name=f"pp{b}_{ci}")
                        nc.vector.tensor_scalar(
                            pt[:, :w], it[:, :w], 1, None,
                            op0=Alu.bitwise_and,
                        )
                        osl = ob[:, base_col + n0:base_col + n0 + w]
                        if ci % 3 == 1:
                            nc.gpsimd.tensor_scalar(
                                osl, pt[:, :w], -2.0, 1.0,
                                op0=Alu.mult, op1=Alu.add,
                            )
                        else:
                            nc.scalar.activation(
                                osl, pt[:, :w], Act.Identity,
                                bias=1.0, scale=-2.0,
                            )
                    nc.sync.dma_start(out=out[bs, :], in_=ob[:])

    nc.compile()
    return nc


def _get_nc(fast: bool):
    key = ("fast" if fast else "full", USE_DR)
    if key not in _CACHE:
        if fast:
            _CACHE[key] = _build(B_LOC, K_MSG, K_PAD, N_BITS - K_MSG, N_BITS,
                                 K_MSG, True, use_dr=USE_DR)
        else:
            _CACHE[key] = _build(B_LOC, K_MSG, K_PAD, N_BITS, N_BITS, 0, False,
                                 use_dr=USE_DR)
    return _CACHE[key]


def _pair_rows(a):
    """[K_PAD, X] -> [K_PAD//2, 2*X]: row t*128+p = concat(a[256t+p], a[256t+128+p])."""
    kp, x = a.shape
    return np.ascontiguousarray(
        a.reshape(kp // 256, 2, P, x).transpose(0, 2, 1, 3).reshape(kp // 2, 2 * x)
    )


def _prep_inputs(m, G, fast: bool):
    """Host-side marshaling: fp8 casts, transposes, padding, DR pairing."""
    mm_dt = _mm_np_dtype()
    if fast:
        g_rows = G[K_MSG:N_BITS]  # parity rows only
    else:
        g_rows = G
    n_par = g_rows.shape[0]
    gT2 = np.zeros((K_PAD, n_par), dtype=mm_dt)
    gT2[:K_MSG] = g_rows.T.astype(mm_dt)  # psum = d (count of set bits)
    if USE_DR:
        gT2 = _pair_rows(gT2)

    m_mm = m.astype(mm_dt)
    in_maps = []
    for c in range(N_CORES):
        m_c = m_mm[c * B_LOC:(c + 1) * B_LOC]
        mT = np.zeros((K_PAD, B_LOC), dtype=mm_dt)
        mT[:K_MSG] = np.ascontiguousarray(m_c.T)
        if USE_DR:
            mT = _pair_rows(mT)
        im = {"mT": mT, "GT2": gT2}
        if fast:
            im["mnat"] = np.ascontiguousarray(m_c)
        in_maps.append(im)
    return in_maps


def _run(m, G, trace=False):
    from concourse.bass_utils import run_bass_kernel_spmd

    binary = bool(((G == 0) | (G == 1)).all())
    if not binary:
        # exact host fallback for arbitrary G (never hit by the LDPC
        # encoder's binary systematic G)
        d = np.mod(m.astype(np.float64) @ G.T.astype(np.float64), 2.0)
        return (1.0 - 2.0 * d).astype(np.float32), None
    fast = bool(np.array_equal(G[:K_MSG], np.eye(K_MSG, dtype=G.dtype)))
    nc = _get_nc(fast)
    in_maps = _prep_inputs(m, G, fast)
    res = run_bass_kernel_spmd(
        nc, in_maps, core_ids=list(range(N_CORES)), trace=trace,
    )
    parts = [res.results[c]["out"] for c in range(N_CORES)]
    full = np.concatenate(parts, axis=0).astype(np.float32)
    return full, res


def kernel(m, G, snr=None):
    m = np.asarray(m)
    G = np.asarray(G)
    full, _ = _run(m, G, trace=False)
    return full
